# revision 49
# baseline (speedup 1.0000x reference)
"""Trainium2 Bass kernel for nn_CrossAttentionLayer (sparse windowed cross-attention).

Math (per batch b):
  q = hidden @ Wq.T + bq ; k = cross @ Wk.T + bk ; v = cross @ Wv.T + bv
  scores = (q k^T) * HD^-0.5 per head, masked to |i-j| <= 64
  attn = softmax(scores) @ v ; attn = attn @ Wo.T + bo
  gate = sigmoid(hidden @ Wg.T + bg) ; attn = gate * attn
  out = LN(0.5*hidden + 0.5*attn) * gamma + beta   (LN is scale-invariant ->
        computed as LN(hidden + gate*attn))

Sharding: data-parallel over batch. 16 sequences -> 8 cores x 2 sequences.
All matmuls bf16 with f32 PSUM accumulation; residual + LayerNorm in f32.

Attention dataflow (per 128-query block, per head):
  scoresT[k, q] = kT_head^T-chunks x qT_head   (3 matmuls, psum [128,3,128])
  probsT = exp(SCALE * scoresT)  (one batched ACT op over a head pair)
  probsT *= maskT01              (banded window mask, multiplicative, DVE)
  attn[q, 0:64], den[q] = probsT^T @ [v | ones]  (v_aug has a ones column)
  attn_sb = attn * (1/den)       (normalize folded into psum->sbuf copy)
Then per token tile: PE-transpose attn (8x 128x128), Wo projection, gated
residual + LayerNorm (rsqrt via Newton iterations on GpSimd to avoid ACT
table switches between Exp and Sqrt).

Device-side layouts per core (T = 1024 tokens = 2 seqs x 512):
  h32  [T, D]  f32   token-major hidden (residual path)
  hT   [D, T]  bf16  hidden transposed (host-pretransposed)
  cT   [D, T]  bf16  cross transposed
  w*T  [D, D]  bf16  transposed weights (in-dim on partitions)
  qT/kT feature-major [D, T]; v_aug/gate token-major
"""

import sys

import numpy as np

sys.path.insert(0, "/opt/trn_rl_repo")

import concourse.bass as bass
import concourse.mybir as mybir
import concourse.tile as tile
from concourse import bacc
from concourse.bass_utils import run_bass_kernel_spmd

import ml_dtypes

F32 = mybir.dt.float32
BF16 = mybir.dt.bfloat16
F8 = mybir.dt.float8e4
U32 = mybir.dt.uint32
AF = mybir.ActivationFunctionType
ALU = mybir.AluOpType
PM_DR = mybir.MatmulPerfMode.DoubleRow

H = 16
D = 1024
HD = 64
S = 512
B = 16
NCORES = 8
SEQ_PER_CORE = B // NCORES      # 2
T = SEQ_PER_CORE * S            # 1024 tokens per core
SCALE = HD ** -0.5
W2 = 64                         # half window
P = 128
NT = T // P                     # 8 token tiles per core
ND = D // P                     # 8 feature chunks
QB = S // P                     # 4 query blocks per sequence
NKT = 3                         # key tiles per query block window (384 keys)
KW = NKT * P
LN_EPS = 1e-5
RSQRT_MAGIC = 0x5F3759DF
NEWTON_ITERS = 1                # seed err ~3.4% -> ~2e-3 after one iteration
USE_NEWTON_RSQRT = True
BUILD_PHASE3 = True
PROBE_VAUG_CONTIG = True
PROBE_SKIP_PV = False
PROBE_SKIP_ATTN = False
VP = 72                         # padded per-head v stride (v | ones | pad)
ND2 = ND // 2                   # fp8 DoubleRow pairs of feature chunks
WS = 8192.0                     # weight scale: xavier weights -> fp8 normals
AS = 256.0                      # attn scale into fp8 normal range
# active key-tile chunks per query block (chunks fully outside the band
# are skipped in scores / exp / PV)
ACTIVE_J = {0: (0, 2), 1: (0, 3), 2: (0, 3), 3: (1, 3)}

_CACHE = {}


def _build_masksT():
    """QB additive mask tiles [P, NKT*P] bf16, transposed.

    maskT[qb][p, j*P+q] = 0 if |(qb*128+q) - (base_kt*128 + j*128 + p)| <= 64
    else -1e5 (so exp(SCALE*(s+mask)) == 0 outside the band).
    """
    m = np.full((QB, P, NKT, P), -1e5, dtype=np.float32)
    for qb in range(QB):
        base_kt = 0 if qb < 2 else 1
        k = base_kt * P + (np.arange(NKT) * P)[None, :, None] + \
            np.arange(P)[:, None, None]
        q = qb * P + np.arange(P)[None, None, :]
        m[qb][np.abs(q - k) <= W2] = 0.0
    return np.ascontiguousarray(m.reshape(QB, P, NKT * P)).astype(
        ml_dtypes.bfloat16)


def _augment_wv(Wv):
    """WvT [D, H*VP]: per head 64 real columns, col 64 zero-weight (bias 1),
    cols 65..VP zero."""
    wvT = np.asarray(Wv, dtype=np.float32).T  # [D, D] = [in, out]
    out = np.zeros((D, H * VP), dtype=np.float32)
    for h in range(H):
        out[:, h * VP:h * VP + HD] = wvT[:, h * HD:(h + 1) * HD]
    return out.astype(ml_dtypes.bfloat16)


def _augment_bv(bv):
    out = np.zeros((H * VP,), dtype=np.float32)
    for h in range(H):
        out[h * VP:h * VP + HD] = np.asarray(bv, dtype=np.float32)[
            h * HD:(h + 1) * HD]
        out[h * VP + HD] = 1.0
    return out


def _build_program(rounds=1, ln_trivial=False):
    nc = bacc.Bacc("TRN2", target_bir_lowering=False, debug=False)

    h32_d = nc.declare_dram_parameter("h32", [T, D], BF16, isOutput=False)
    # fp8 DoubleRow operands: [pair, 128, 2, X] where [:, :, i, :] is feature
    # chunk 2*pair+i. Weights are pre-scaled by WS host-side so xavier-sized
    # values land in fp8-normal range; psum results are rescaled by 1/WS.
    h8_d = nc.declare_dram_parameter("h8", [ND2, P, 2, T], F8, isOutput=False)
    c8_d = nc.declare_dram_parameter("c8", [ND2, P, 2, T], F8, isOutput=False)
    wq8_d = nc.declare_dram_parameter("wq8", [ND2, P, 2, D], F8, isOutput=False)
    wk8_d = nc.declare_dram_parameter("wk8", [ND2, P, 2, D], F8, isOutput=False)
    # wvT augmented host-side to [D, H*VP]: per head 64 value cols + a
    # zero-weight col whose bias is 1.0 (emits the softmax-denominator ones)
    wv8_d = nc.declare_dram_parameter("wv8", [ND2, P, 2, H * VP], F8,
                                      isOutput=False)
    wg8_d = nc.declare_dram_parameter("wg8", [ND2, P, 2, D], F8, isOutput=False)
    wo8_d = nc.declare_dram_parameter("wo8", [ND2, P, 2, D], F8, isOutput=False)
    bqs_d = nc.declare_dram_parameter("bqs", [P, ND], F32, isOutput=False)
    bks_d = nc.declare_dram_parameter("bks", [P, ND], F32, isOutput=False)
    bv_d = nc.declare_dram_parameter("bv", [H * VP], F32, isOutput=False)
    bo_d = nc.declare_dram_parameter("bo", [D], F32, isOutput=False)
    gamma_d = nc.declare_dram_parameter("gamma", [D], F32, isOutput=False)
    beta_d = nc.declare_dram_parameter("beta", [D], F32, isOutput=False)
    masksT_d = nc.declare_dram_parameter("masksT", [QB, P, NKT * P], BF16,
                                         isOutput=False)
    ident_d = nc.declare_dram_parameter("ident", [P, P], BF16, isOutput=False)
    bgw_d = nc.declare_dram_parameter("bgw", [1, D], BF16, isOutput=False)
    out_d = nc.declare_dram_parameter("out", [T, D], F32, isOutput=True)

    def bcast(vec_d):
        # [D] dram vector -> [P, D] AP with 0-stride partition dim (DMA broadcast)
        a = vec_d[:]
        return bass.AP(tensor=a.tensor, offset=a.offset, ap=[[0, P], *a.ap])

    with tile.TileContext(nc) as tc:
        from contextlib import ExitStack, nullcontext

        # On-device benchmarking loop: rounds > 1 repeats the FULL kernel
        # (including every input DMA) back to back so per-round time can be
        # measured without the multi-ms per-dispatch tunnel overhead.
        loop_ctx = tc.For_i(0, rounds, 1) if rounds > 1 else nullcontext()
        with loop_ctx, ExitStack() as ctx:
            consts = ctx.enter_context(tc.tile_pool(name="consts", bufs=1))
            persist = ctx.enter_context(tc.tile_pool(name="persist", bufs=1))
            work = ctx.enter_context(tc.tile_pool(name="work", bufs=2))

            # ---- constants ----
            masksT_sb = []
            for qb in range(QB):
                m = consts.tile([P, NKT * P], BF16, tag=f"maskT{qb}",
                                name=f"maskT{qb}")
                nc.sync.dma_start(out=m, in_=masksT_d[qb])
                masksT_sb.append(m)
            ident = consts.tile([P, P], BF16, tag="ident", name="ident")
            nc.sync.dma_start(out=ident, in_=ident_d[:])
            bqs = consts.tile([P, ND], F32, tag="bqs", name="bqs")
            nc.sync.dma_start(out=bqs, in_=bqs_d[:])
            bks = consts.tile([P, ND], F32, tag="bks", name="bks")
            nc.sync.dma_start(out=bks, in_=bks_d[:])
            bv_bc = consts.tile([P, H * VP], F32, tag="bv_bc", name="bv_bc")
            nc.sync.dma_start(out=bv_bc, in_=bcast(bv_d))
            bo_bc = consts.tile([P, D], F32, tag="bo_bc", name="bo_bc")
            nc.sync.dma_start(out=bo_bc, in_=bcast(bo_d))
            gamma_bc = consts.tile([P, D], F32, tag="gamma_bc", name="gamma_bc")
            nc.sync.dma_start(out=gamma_bc, in_=bcast(gamma_d))
            beta_bc = consts.tile([P, D], F32, tag="beta_bc", name="beta_bc")
            nc.sync.dma_start(out=beta_bc, in_=bcast(beta_d))
            ones_col = consts.tile([1, P], BF16, tag="ones_col",
                                   name="ones_col")
            nc.vector.memset(ones_col, 1.0)
            bgw_sb = consts.tile([1, D], BF16, tag="bgw", name="bgw")
            nc.sync.dma_start(out=bgw_sb, in_=bgw_d[:])
            magic_u = consts.tile([P, 1], U32, tag="magic", name="magic")
            if USE_NEWTON_RSQRT:
                nc.vector.memset(magic_u, RSQRT_MAGIC)
            eps_sb = consts.tile([P, 1], F32, tag="eps", name="eps")
            nc.vector.memset(eps_sb, LN_EPS)

            # ---- persistent activation tensors ----
            kT = [persist.tile([P, T], BF16, tag=f"kT{i}", name=f"kT{i}")
                  for i in range(ND)]
            # inner dim padded to VP=72 so each head slice starts 16B-aligned
            v_aug = [persist.tile([P, H, VP], BF16, tag=f"v{i}",
                                  name=f"v{i}") for i in range(NT)]
            qT = [persist.tile([P, T], BF16, tag=f"qT{i}", name=f"qT{i}")
                  for i in range(ND)]
            gate = [persist.tile([P, D], BF16, tag=f"g{i}", name=f"g{i}")
                    for i in range(NT)]

            # ---- input streaming: ALL major input DMAs issued up front so
            # PE never stalls on a pool-close barrier between phases.
            # pinA (cT, wkT, wvT) closes after phase 1 so woT can stream into
            # its space during Q-proj; pinB (hT, wqT) and pg (wgT) close
            # after the gate.
            # pool-stack (LIFO) order: pinA opened last of the input pools so
            # it can close right after phase 1; pinB/pg stay open to the end.
            pinB = ctx.enter_context(tc.tile_pool(name="pinB", bufs=1))
            pg = ctx.enter_context(tc.tile_pool(name="pg", bufs=1))
            pinA_ctx = tc.tile_pool(name="pinA", bufs=1)
            pinA = pinA_ctx.__enter__()

            c8_sb, wk8_sb, wv8_sb = [], [], []
            h8_sb, wq8_sb, wg8_sb = [], [], []
            # cT + wkT first: the kT matmuls need only these. Issued from the
            # (otherwise idle) Pool sequencer — SP's per-DMA config time
            # (~565 ns) serializes the stream badly when it issues all of
            # them itself.
            for p2 in range(ND2):
                t_ = pinA.tile([P, 2, T], F8, tag=f"c8_{p2}", name=f"c8_{p2}")
                nc.gpsimd.dma_start(out=t_, in_=c8_d[p2])
                c8_sb.append(t_)
                t_ = pinA.tile([P, 2, D], F8, tag=f"wk8_{p2}", name=f"wk8_{p2}")
                nc.gpsimd.dma_start(out=t_, in_=wk8_d[p2])
                wk8_sb.append(t_)
            for p2 in range(ND2):
                t_ = pinA.tile([P, 2, H * VP], F8, tag=f"wv8_{p2}",
                               name=f"wv8_{p2}")
                nc.sync.dma_start(out=t_, in_=wv8_d[p2])
                wv8_sb.append(t_)
                t_ = pinB.tile([P, 2, T], F8, tag=f"h8_{p2}", name=f"h8_{p2}")
                nc.sync.dma_start(out=t_, in_=h8_d[p2])
                h8_sb.append(t_)
            for p2 in range(ND2):
                t_ = pinB.tile([P, 2, D], F8, tag=f"wq8_{p2}", name=f"wq8_{p2}")
                nc.scalar.dma_start(out=t_, in_=wq8_d[p2])
                wq8_sb.append(t_)
                t_ = pg.tile([P, 2, D], F8, tag=f"wg8_{p2}", name=f"wg8_{p2}")
                nc.scalar.dma_start(out=t_, in_=wg8_d[p2])
                wg8_sb.append(t_)

            # ================= phase 1: K, V from cross =================
            ps12_ctx = tc.tile_pool(name="ps12", bufs=1, space="PSUM")
            ps12 = ps12_ctx.__enter__()

            for oc in range(ND):
                ps = ps12.tile([P, 2, 512], F32, tag="proj2", bufs=2,
                               name="ps_k")
                for th in range(2):
                    for p2 in range(ND2):
                        nc.tensor.matmul(
                            ps[:, th, :],
                            lhsT=wk8_sb[p2][:, :, oc * P:(oc + 1) * P],
                            rhs=c8_sb[p2][:, :, th * 512:(th + 1) * 512],
                            start=(p2 == 0), stop=(p2 == ND2 - 1),
                            perf_mode=PM_DR,
                        )
                nc.vector.tensor_scalar(
                    out=kT[oc], in0=ps.rearrange("p a b -> p (a b)"),
                    scalar1=1.0 / WS, scalar2=bks[:, oc:oc + 1],
                    op0=ALU.mult, op1=ALU.add,
                )

            # v_aug projection: 4 heads per matmul group (N = 4*VP = 288),
            # every elementwise op contiguous
            NVG = 4 * VP  # 288
            for tt in range(NT):
                for qg in range(4):
                    ps = ps12.tile([P, 512], F32, tag="proj", bufs=2,
                                   name="ps_v")
                    for p2 in range(ND2):
                        nc.tensor.matmul(
                            ps[:, 0:NVG],
                            lhsT=c8_sb[p2][:, :, tt * P:(tt + 1) * P],
                            rhs=wv8_sb[p2][:, :, qg * NVG:(qg + 1) * NVG],
                            start=(p2 == 0), stop=(p2 == ND2 - 1),
                            perf_mode=PM_DR,
                        )
                    nc.vector.scalar_tensor_tensor(
                        out=v_aug[tt][:, qg * 4:(qg + 1) * 4, :].rearrange(
                            "p a b -> p (a b)"),
                        in0=ps[:, 0:NVG], scalar=1.0 / WS,
                        in1=bv_bc[:, qg * NVG:(qg + 1) * NVG],
                        op0=ALU.mult, op1=ALU.add,
                    )

            # phase 1 inputs dead -> free their space; woT streams into it
            # while Q-proj runs
            pinA_ctx.__exit__(None, None, None)
            po = ctx.enter_context(tc.tile_pool(name="po", bufs=1))
            wo8_sb = []
            for p2 in range(ND2):
                t_ = po.tile([P, 2, D], F8, tag=f"wo8_{p2}", name=f"wo8_{p2}")
                nc.gpsimd.dma_start(out=t_, in_=wo8_d[p2])
                wo8_sb.append(t_)

            # ============ phase 2: Q, gate from hidden ============
            for oc in range(ND):
                ps = ps12.tile([P, 2, 512], F32, tag="proj2", bufs=2,
                               name="ps_q")
                for th in range(2):
                    for p2 in range(ND2):
                        nc.tensor.matmul(
                            ps[:, th, :],
                            lhsT=wq8_sb[p2][:, :, oc * P:(oc + 1) * P],
                            rhs=h8_sb[p2][:, :, th * 512:(th + 1) * 512],
                            start=(p2 == 0), stop=(p2 == ND2 - 1),
                            perf_mode=PM_DR,
                        )
                nc.vector.tensor_scalar(
                    out=qT[oc], in0=ps.rearrange("p a b -> p (a b)"),
                    scalar1=1.0 / WS, scalar2=bqs[:, oc:oc + 1],
                    op0=ALU.mult, op1=ALU.add,
                )

            for tt in range(NT):
                ps = ps12.tile([P, 2, 512], F32, tag="proj2", bufs=2,
                               name="ps_g")
                for oh in range(2):
                    for p2 in range(ND2):
                        nc.tensor.matmul(
                            ps[:, oh, :],
                            lhsT=h8_sb[p2][:, :, tt * P:(tt + 1) * P],
                            rhs=wg8_sb[p2][:, :, oh * 512:(oh + 1) * 512],
                            start=(p2 == 0), stop=False,
                            perf_mode=PM_DR,
                        )
                    # bias via rank-1 matmul (ones ⊗ WS*bg) so sigmoid can
                    # read the psum directly — saves a DVE pass
                    nc.tensor.matmul(
                        ps[:, oh, :],
                        lhsT=ones_col,
                        rhs=bgw_sb[:, oh * 512:(oh + 1) * 512],
                        start=False, stop=True, skip_group_check=True,
                    )
                nc.scalar.activation(
                    out=gate[tt], in_=ps.rearrange("p a b -> p (a b)"),
                    func=AF.Sigmoid, scale=1.0 / WS,
                )

            ps12_ctx.__exit__(None, None, None)

            # big phase-3 work tiles (fresh space; pinA's is used by po/w3)
            w3 = ctx.enter_context(tc.tile_pool(name="w3", bufs=3))

            # ===== phase 3: attention + out proj + epilogue =====
            with tc.tile_pool(name="ps3", bufs=1, space="PSUM") as ps3:
                if not BUILD_PHASE3:
                    for tt in range(NT):
                        h32t = w3.tile([P, D], F32, tag="h32t", name="h32t")
                        nc.sync.dma_start(out=h32t,
                                          in_=h32_d[tt * P:(tt + 1) * P, :])
                        ob = w3.tile([P, D], F32, tag="ob", name="ob")
                        nc.vector.tensor_add(out=ob, in0=h32t, in1=gate[tt])
                        nc.sync.dma_start(out=out_d[tt * P:(tt + 1) * P, :],
                                          in_=ob)
                    pass
                else:
                    for tt in range(NT):
                        s = tt // QB
                        qb = tt % QB
                        base_kt = (0 if qb < 2 else 1) + s * QB  # global key tile
                        # active window chunks: qb0's chunk 2 and qb3's chunk 0
                        # are entirely outside the band -> skip them everywhere
                        j0, j1 = ACTIVE_J[qb]
                        nj = j1 - j0

                        attn_sb = w3.tile([P, H, HD], BF16, tag="attn_sb",
                                            name=f"attn_sb{tt}")
                        for c in range(ND):
                            # scoresT for both heads of the pair: [k, hh, j, q]
                            # (inner dim 512 so each head slice is bank-aligned
                            # and the exp read stays within a single bank)
                            ps_sc = ps3.tile([P, 2, 512], F32, tag="sc", bufs=2,
                                             name="ps_sc")
                            for u in range(2):
                                h = 2 * c + u
                                row0 = (h % 2) * HD
                                for j in range(j0, j1):
                                    nc.tensor.matmul(
                                        ps_sc[:, u, j * P:(j + 1) * P],
                                        lhsT=kT[c][row0:row0 + HD,
                                                   (base_kt + j) * P:
                                                   (base_kt + j + 1) * P],
                                        rhs=qT[c][row0:row0 + HD,
                                                  tt * P:(tt + 1) * P],
                                        start=(j == j0), stop=False,
                                    )
                                # accumulate the additive band mask on PE:
                                # ident.T @ maskAddT == maskAddT
                                nc.tensor.matmul(
                                    ps_sc[:, u, j0 * P:j1 * P],
                                    lhsT=ident,
                                    rhs=masksT_sb[qb][:, j0 * P:j1 * P],
                                    start=False, stop=True,
                                    skip_group_check=True,
                                )
                            probsT = w3.tile([P, 2, NKT, P], BF16, tag="probsT",
                                               name="probsT", bufs=3)
                            # one exp per chunk over both heads: strided psum
                            # read crossing the bank boundary (ScE psum reads
                            # allow up to 4K free)
                            nc.scalar.activation(
                                out=probsT[:, :, j0:j1, :].rearrange(
                                    "p a b c -> p a (b c)"),
                                in_=ps_sc[:, :, j0 * P:j1 * P],
                                func=AF.Exp, scale=SCALE)
                            # attn + denominator via v_aug ones column.
                            # one PSUM tile per head: PE-write of head u=1 must
                            # not share a bank with DVE reads of head u=0
                            # (PSUM bank collisions are a hardware abort)
                            # both heads' PV into ONE psum bank ([P,2,VP]
                            # fits 576B < 2KB) so a single strided reciprocal
                            # covers both denominators. DVE reads wait for
                            # both PV groups (same bank as PE writes).
                            rden = work.tile([P, 2], F32, tag="rden", name="rden")
                            ps_aT = ps3.tile([P, 2, VP], F32, tag="aT",
                                             bufs=2, name="ps_aT")
                            for u in range(2):
                                h = 2 * c + u
                                for j in range(j0, j1):
                                    nc.tensor.matmul(
                                        ps_aT[:, u, 0:HD + 1],
                                        lhsT=probsT[:, u, j, :],
                                        rhs=v_aug[base_kt + j][:, h, 0:HD + 1],
                                        start=(j == j0), stop=(j == j1 - 1),
                                    )
                            nc.vector.reciprocal(out=rden,
                                                 in_=ps_aT[:, :, HD])
                            for u in range(2):
                                nc.vector.tensor_scalar_mul(
                                    out=attn_sb[:, 2 * c + u, :],
                                    in0=ps_aT[:, u, 0:HD],
                                    scalar1=rden[:, u:u + 1],
                                )

                        # transpose attn to feature-major (fp8, scaled by AS
                        # into fp8-normal range) for the Wo projection; the
                        # psum->sbuf copies ride the ACT engine, DVE is the
                        # attention-phase bottleneck
                        attnT = w3.tile([P, ND, P], F8, tag="attnT",
                                        name=f"attnT{tt}")
                        for c2 in range(ND // 2):
                            # two 128x128 transposes into one psum bank, one
                            # ACT copy for both
                            ps_tp = ps3.tile([P, 2, P], F32, tag="tp", bufs=2,
                                             name="ps_tp")
                            for w in range(2):
                                c = 2 * c2 + w
                                nc.tensor.matmul(
                                    ps_tp[:, w, :],
                                    lhsT=attn_sb[:, 2 * c:2 * c + 2, :],
                                    rhs=ident, start=True, stop=True,
                                )
                            nc.scalar.activation(
                                out=attnT[:, 2 * c2:2 * c2 + 2, :].rearrange(
                                    "p a b -> p (a b)"),
                                in_=ps_tp.rearrange("p a b -> p (a b)"),
                                func=AF.Identity, scale=AS)

                        # out projection + epilogue for this token tile
                        h32t = w3.tile([P, D], BF16, tag="h32t", name="h32t")
                        nc.sync.dma_start(out=h32t, in_=h32_d[tt * P:(tt + 1) * P, :])
                        ta = w3.tile([P, D], F32, tag="ta", name="ta")
                        for oh in range(2):
                            ps_o = ps3.tile([P, 512], F32, tag="sc", bufs=2,
                                            name="ps_o")
                            for p2 in range(ND2):
                                nc.tensor.matmul(
                                    ps_o,
                                    lhsT=attnT[:, 2 * p2:2 * p2 + 2, :],
                                    rhs=wo8_sb[p2][:, :, oh * 512:(oh + 1) * 512],
                                    start=(p2 == 0), stop=(p2 == ND2 - 1),
                                    perf_mode=PM_DR,
                                )
                            if ln_trivial:
                                # bo == 0 under the same trivial-affine init
                                nc.vector.tensor_scalar_mul(
                                    out=ta[:, oh * 512:(oh + 1) * 512],
                                    in0=ps_o, scalar1=1.0 / (WS * AS),
                                )
                            else:
                                nc.vector.scalar_tensor_tensor(
                                    out=ta[:, oh * 512:(oh + 1) * 512],
                                    in0=ps_o, scalar=1.0 / (WS * AS),
                                    in1=bo_bc[:, oh * 512:(oh + 1) * 512],
                                    op0=ALU.mult, op1=ALU.add,
                                )
                        # gated residual: pre = hidden + gate*attn (LN scale-inv)
                        tb = w3.tile([P, D], F32, tag="tb", name="tb")
                        nc.gpsimd.tensor_mul(out=ta, in0=ta, in1=gate[tt])
                        nc.gpsimd.tensor_add(out=tb, in0=ta, in1=h32t)
                        # LayerNorm stats
                        stats = work.tile([P, 2, 6], F32, tag="stats", name="stats")
                        for half in range(2):
                            nc.vector.bn_stats(out=stats[:, half, :],
                                               in_=tb[:, half * 512:(half + 1) * 512])
                        mv = work.tile([P, 2], F32, tag="mv", name="mv")
                        nc.vector.bn_aggr(out=mv, in_=stats)
                        # rstd = rsqrt(var + eps) via Newton on GpSimd (keeps the
                        # ACT engine's Exp table resident)
                        if USE_NEWTON_RSQRT:
                            xe = work.tile([P, 1], F32, tag="xe", name="xe")
                            nc.vector.tensor_scalar_add(out=xe, in0=mv[:, 1:2],
                                                        scalar1=LN_EPS)
                            yy = work.tile([P, 1], F32, tag="yy", name="yy")
                            tmp_u = work.tile([P, 1], U32, tag="tmp_u",
                                              name="tmp_u")
                            nc.vector.tensor_scalar(
                                out=tmp_u, in0=xe.bitcast(U32), scalar1=1,
                                scalar2=None, op0=ALU.logical_shift_right,
                            )
                            nc.vector.tensor_sub(out=yy.bitcast(U32),
                                                 in0=magic_u, in1=tmp_u)
                            t1 = work.tile([P, 1], F32, tag="nt1", name="nt1")
                            for _ in range(NEWTON_ITERS):
                                nc.vector.tensor_mul(out=t1, in0=yy, in1=yy)
                                nc.vector.tensor_mul(out=t1, in0=t1, in1=xe)
                                nc.vector.tensor_scalar(
                                    out=t1, in0=t1, scalar1=-0.5, scalar2=1.5,
                                    op0=ALU.mult, op1=ALU.add,
                                )
                                nc.vector.tensor_mul(out=yy, in0=yy, in1=t1)
                        else:
                            yy = work.tile([P, 1], F32, tag="yy", name="yy")
                            nc.scalar.activation(out=yy, in_=mv[:, 1:2],
                                                 func=AF.Sqrt, bias=eps_sb,
                                                 scale=1.0)
                            nc.vector.reciprocal(out=yy, in_=yy)
                        if ln_trivial:
                            # gamma==1, beta==0 (detected host-side):
                            # out = (tb - mean) * rstd in ONE fused DVE op
                            nc.vector.tensor_scalar(
                                out=ta, in0=tb, scalar1=mv[:, 0:1],
                                scalar2=yy, op0=ALU.subtract, op1=ALU.mult,
                            )
                            nc.sync.dma_start(
                                out=out_d[tt * P:(tt + 1) * P, :], in_=ta)
                        else:
                            # (tb-mean)*gamma -> ta; *rstd; + beta -> tb
                            nc.vector.scalar_tensor_tensor(
                                out=ta, in0=tb, scalar=mv[:, 0:1], in1=gamma_bc,
                                op0=ALU.subtract, op1=ALU.mult,
                            )
                            nc.vector.tensor_scalar_mul(out=ta, in0=ta,
                                                        scalar1=yy)
                            nc.gpsimd.tensor_add(out=tb, in0=ta, in1=beta_bc)
                            nc.sync.dma_start(
                                out=out_d[tt * P:(tt + 1) * P, :], in_=tb)

    nc.compile()
    return nc


def _pair8(M, scale=1.0):
    """[D, X] f32 -> fp8 DoubleRow layout [ND2, P, 2, X]."""
    X = M.shape[1]
    return np.ascontiguousarray(
        (np.asarray(M, np.float32) * scale).reshape(ND2, 2, P, X)
        .transpose(0, 2, 1, 3)).astype(ml_dtypes.float8_e4m3)


def _prep_host(inputs):
    bf = ml_dtypes.bfloat16
    hidden = np.ascontiguousarray(inputs["hidden_states"], dtype=np.float32)
    cross = np.ascontiguousarray(inputs["cross_states"], dtype=np.float32)
    wvT_aug = np.asarray(_augment_wv(inputs["Wv"]), dtype=np.float32)
    shared = {
        "wq8": _pair8(inputs["Wq"].T, WS),
        "wk8": _pair8(inputs["Wk"].T, WS),
        "wv8": _pair8(wvT_aug, WS),
        "wg8": _pair8(inputs["Wg"].T, WS),
        "wo8": _pair8(inputs["Wo"].T, WS),
        "bqs": np.ascontiguousarray(
            inputs["bq"].astype(np.float32).reshape(ND, P).T),
        "bks": np.ascontiguousarray(
            inputs["bk"].astype(np.float32).reshape(ND, P).T),
        "bv": _augment_bv(inputs["bv"]),
        "bgw": (inputs["bg"].astype(np.float32) * WS).reshape(1, D).astype(bf),
        "bo": inputs["bo"].astype(np.float32),
        "gamma": inputs["gamma"].astype(np.float32),
        "beta": inputs["beta"].astype(np.float32),
        "masksT": _build_masksT(),
        "ident": np.eye(P, dtype=bf),
    }
    in_maps = []
    for core in range(NCORES):
        hs = hidden[core * SEQ_PER_CORE:(core + 1) * SEQ_PER_CORE].reshape(T, D)
        cs = cross[core * SEQ_PER_CORE:(core + 1) * SEQ_PER_CORE].reshape(T, D)
        m = dict(shared)
        m["h32"] = np.ascontiguousarray(hs).astype(bf)
        m["h8"] = _pair8(hs.T)
        m["c8"] = _pair8(cs.T)
        in_maps.append(m)
    return in_maps


def _ln_trivial(inputs):
    return (np.allclose(np.asarray(inputs["gamma"], np.float32), 1.0) and
            np.allclose(np.asarray(inputs["beta"], np.float32), 0.0) and
            np.allclose(np.asarray(inputs["bo"], np.float32), 0.0))


def _run(inputs, trace=False):
    lt = _ln_trivial(inputs)
    key = f"nc1_{lt}"
    if key not in _CACHE:
        _CACHE[key] = _build_program(rounds=1, ln_trivial=lt)
    nc = _CACHE[key]
    in_maps = _prep_host(inputs)
    res = run_bass_kernel_spmd(nc, in_maps, list(range(NCORES)), trace=trace)
    out = np.empty((B, S, D), dtype=np.float32)
    for core in range(NCORES):
        out[core * SEQ_PER_CORE:(core + 1) * SEQ_PER_CORE] = (
            np.asarray(res.results[core]["out"], dtype=np.float32).reshape(
                SEQ_PER_CORE, S, D))
    return out, res


def kernel(**inputs):
    out, _ = _run(inputs, trace=False)
    return out


def bench(inputs, iters=20, rounds=1):
    """Amortized device-time benchmark: device-resident inputs, N back-to-back
    dispatches of a program that runs the full kernel `rounds` times
    on-device, report per-round wall time."""
    import time

    import jax
    from jax.sharding import Mesh, NamedSharding, PartitionSpec
    from jax.experimental.shard_map import shard_map
    from concourse import bass2jax, mybir as _mybir

    lt = _ln_trivial(inputs)
    key = f"nc{rounds}_{lt}"
    if key not in _CACHE:
        _CACHE[key] = _build_program(rounds=rounds, ln_trivial=lt)
    nc = _CACHE[key]
    in_maps = _prep_host(inputs)
    bass2jax.install_neuronx_cc_hook()

    partition_name = (nc.partition_id_tensor.name if nc.partition_id_tensor
                      else None)
    in_names, out_names, out_avals, zero_outs = [], [], [], []
    for alloc in nc.m.functions[0].allocations:
        if not isinstance(alloc, _mybir.MemoryLocationSet):
            continue
        name = alloc.memorylocations[0].name
        if alloc.kind == "ExternalInput":
            if name != partition_name:
                in_names.append(name)
        elif alloc.kind == "ExternalOutput":
            out_names.append(name)
            shape = tuple(alloc.tensor_shape)
            dtype = _mybir.dt.np(alloc.dtype)
            out_avals.append(jax.core.ShapedArray(shape, dtype))
            zero_outs.append(np.zeros(shape, dtype))
    n_params = len(in_names)
    all_in_names = in_names + out_names
    if partition_name is not None:
        all_in_names.append(partition_name)

    def _body(*args):
        operands = list(args)
        if partition_name is not None:
            operands.append(bass2jax.partition_id_tensor())
        outs = bass2jax._bass_exec_p.bind(
            *operands,
            out_avals=tuple(out_avals),
            in_names=tuple(all_in_names),
            out_names=tuple(out_names),
            lowering_input_output_aliases=(),
            sim_require_finite=True,
            sim_require_nnan=True,
            nc=nc,
        )
        return tuple(outs)

    devices = jax.devices()[:NCORES]
    mesh = Mesh(np.asarray(devices), ("core",))
    spec = PartitionSpec("core")
    n_outs = len(out_names)
    sharded = jax.jit(
        shard_map(_body, mesh=mesh, in_specs=(spec,) * (n_params + n_outs),
                  out_specs=(spec,) * n_outs, check_rep=False),
        keep_unused=True,
    )
    concat_in = [
        np.concatenate([np.asarray(in_maps[c][name]) for c in range(NCORES)],
                       axis=0)
        for name in in_names
    ]
    concat_zero = [np.zeros((NCORES * z.shape[0], *z.shape[1:]), z.dtype)
                   for z in zero_outs]
    sh = NamedSharding(mesh, spec)
    dev_in = [jax.device_put(a, sh) for a in concat_in]
    dev_zero = [jax.device_put(a, sh) for a in concat_zero]

    # warmup (compile)
    out = sharded(*dev_in, *dev_zero)
    jax.block_until_ready(out)
    t0 = time.perf_counter()
    for _ in range(iters):
        out = sharded(*dev_in, *dev_zero)
    jax.block_until_ready(out)
    t1 = time.perf_counter()
    per_round_ns = (t1 - t0) / (iters * rounds) * 1e9
    return per_round_ns, out



# revision 50
# speedup vs baseline: 1.0369x; 1.0369x over previous
"""Trainium2 Bass kernel for nn_CrossAttentionLayer (sparse windowed cross-attention).

Math (per batch b):
  q = hidden @ Wq.T + bq ; k = cross @ Wk.T + bk ; v = cross @ Wv.T + bv
  scores = (q k^T) * HD^-0.5 per head, masked to |i-j| <= 64
  attn = softmax(scores) @ v ; attn = attn @ Wo.T + bo
  gate = sigmoid(hidden @ Wg.T + bg) ; attn = gate * attn
  out = LN(0.5*hidden + 0.5*attn) * gamma + beta   (LN is scale-invariant ->
        computed as LN(hidden + gate*attn))

Sharding: data-parallel over batch. 16 sequences -> 8 cores x 2 sequences.
All matmuls bf16 with f32 PSUM accumulation; residual + LayerNorm in f32.

Attention dataflow (per 128-query block, per head):
  scoresT[k, q] = kT_head^T-chunks x qT_head   (3 matmuls, psum [128,3,128])
  probsT = exp(SCALE * scoresT)  (one batched ACT op over a head pair)
  probsT *= maskT01              (banded window mask, multiplicative, DVE)
  attn[q, 0:64], den[q] = probsT^T @ [v | ones]  (v_aug has a ones column)
  attn_sb = attn * (1/den)       (normalize folded into psum->sbuf copy)
Then per token tile: PE-transpose attn (8x 128x128), Wo projection, gated
residual + LayerNorm (rsqrt via Newton iterations on GpSimd to avoid ACT
table switches between Exp and Sqrt).

Device-side layouts per core (T = 1024 tokens = 2 seqs x 512):
  h32  [T, D]  f32   token-major hidden (residual path)
  hT   [D, T]  bf16  hidden transposed (host-pretransposed)
  cT   [D, T]  bf16  cross transposed
  w*T  [D, D]  bf16  transposed weights (in-dim on partitions)
  qT/kT feature-major [D, T]; v_aug/gate token-major
"""

import sys

import numpy as np

sys.path.insert(0, "/opt/trn_rl_repo")

import concourse.bass as bass
import concourse.mybir as mybir
import concourse.tile as tile
from concourse import bacc
from concourse.bass_utils import run_bass_kernel_spmd

import ml_dtypes

F32 = mybir.dt.float32
BF16 = mybir.dt.bfloat16
F8 = mybir.dt.float8e4
U32 = mybir.dt.uint32
AF = mybir.ActivationFunctionType
ALU = mybir.AluOpType
PM_DR = mybir.MatmulPerfMode.DoubleRow

H = 16
D = 1024
HD = 64
S = 512
B = 16
NCORES = 8
SEQ_PER_CORE = B // NCORES      # 2
T = SEQ_PER_CORE * S            # 1024 tokens per core
SCALE = HD ** -0.5
W2 = 64                         # half window
P = 128
NT = T // P                     # 8 token tiles per core
ND = D // P                     # 8 feature chunks
QB = S // P                     # 4 query blocks per sequence
NKT = 3                         # key tiles per query block window (384 keys)
KW = NKT * P
LN_EPS = 1e-5
RSQRT_MAGIC = 0x5F3759DF
NEWTON_ITERS = 1                # seed err ~3.4% -> ~2e-3 after one iteration
USE_NEWTON_RSQRT = True
BUILD_PHASE3 = True
PROBE_VAUG_CONTIG = True
PROBE_SKIP_PV = False
PROBE_SKIP_ATTN = False
VP = 72                         # padded per-head v stride (v | ones | pad)
ND2 = ND // 2                   # fp8 DoubleRow pairs of feature chunks
WS = 8192.0                     # weight scale: xavier weights -> fp8 normals
AS = 256.0                      # attn scale into fp8 normal range
# active key-tile chunks per query block (chunks fully outside the band
# are skipped in scores / exp / PV)
ACTIVE_J = {0: (0, 2), 1: (0, 3), 2: (0, 3), 3: (1, 3)}

_CACHE = {}


def _build_masksT():
    """QB additive mask tiles [P, NKT*P] bf16, transposed.

    maskT[qb][p, j*P+q] = 0 if |(qb*128+q) - (base_kt*128 + j*128 + p)| <= 64
    else -1e5 (so exp(SCALE*(s+mask)) == 0 outside the band).
    """
    m = np.full((QB, P, NKT, P), -1e5, dtype=np.float32)
    for qb in range(QB):
        base_kt = 0 if qb < 2 else 1
        k = base_kt * P + (np.arange(NKT) * P)[None, :, None] + \
            np.arange(P)[:, None, None]
        q = qb * P + np.arange(P)[None, None, :]
        m[qb][np.abs(q - k) <= W2] = 0.0
    return np.ascontiguousarray(m.reshape(QB, P, NKT * P)).astype(
        ml_dtypes.bfloat16)


def _augment_wv(Wv):
    """WvT [D, H*VP]: per head 64 real columns, col 64 zero-weight (bias 1),
    cols 65..VP zero."""
    wvT = np.asarray(Wv, dtype=np.float32).T  # [D, D] = [in, out]
    out = np.zeros((D, H * VP), dtype=np.float32)
    for h in range(H):
        out[:, h * VP:h * VP + HD] = wvT[:, h * HD:(h + 1) * HD]
    return out.astype(ml_dtypes.bfloat16)


def _augment_bv(bv):
    out = np.zeros((H * VP,), dtype=np.float32)
    for h in range(H):
        out[h * VP:h * VP + HD] = np.asarray(bv, dtype=np.float32)[
            h * HD:(h + 1) * HD]
        out[h * VP + HD] = 1.0
    return out


def _build_program(rounds=1, ln_trivial=False):
    nc = bacc.Bacc("TRN2", target_bir_lowering=False, debug=False)

    h32_d = nc.declare_dram_parameter("h32", [T, D], BF16, isOutput=False)
    # fp8 DoubleRow operands: [pair, 128, 2, X] where [:, :, i, :] is feature
    # chunk 2*pair+i. Weights are pre-scaled by WS host-side so xavier-sized
    # values land in fp8-normal range; psum results are rescaled by 1/WS.
    h8_d = nc.declare_dram_parameter("h8", [ND2, P, 2, T], F8, isOutput=False)
    c8_d = nc.declare_dram_parameter("c8", [ND2, P, 2, T], F8, isOutput=False)
    wq8_d = nc.declare_dram_parameter("wq8", [ND2, P, 2, D], F8, isOutput=False)
    wk8_d = nc.declare_dram_parameter("wk8", [ND2, P, 2, D], F8, isOutput=False)
    # wvT augmented host-side to [D, H*VP]: per head 64 value cols + a
    # zero-weight col whose bias is 1.0 (emits the softmax-denominator ones)
    wv8_d = nc.declare_dram_parameter("wv8", [ND2, P, 2, H * VP], F8,
                                      isOutput=False)
    wg8_d = nc.declare_dram_parameter("wg8", [ND2, P, 2, D], F8, isOutput=False)
    wo8_d = nc.declare_dram_parameter("wo8", [ND2, P, 2, D], F8, isOutput=False)
    bqs_d = nc.declare_dram_parameter("bqs", [P, ND], F32, isOutput=False)
    bks_d = nc.declare_dram_parameter("bks", [P, ND], F32, isOutput=False)
    bv_d = nc.declare_dram_parameter("bv", [H * VP], BF16, isOutput=False)
    bo_d = nc.declare_dram_parameter("bo", [D], F32, isOutput=False)
    gamma_d = nc.declare_dram_parameter("gamma", [D], F32, isOutput=False)
    beta_d = nc.declare_dram_parameter("beta", [D], F32, isOutput=False)
    masksT_d = nc.declare_dram_parameter("masksT", [QB, P, NKT * P], BF16,
                                         isOutput=False)
    ident_d = nc.declare_dram_parameter("ident", [P, P], BF16, isOutput=False)
    bgw_d = nc.declare_dram_parameter("bgw", [1, D], BF16, isOutput=False)
    out_d = nc.declare_dram_parameter("out", [T, D], F32, isOutput=True)

    def bcast(vec_d):
        # [D] dram vector -> [P, D] AP with 0-stride partition dim (DMA broadcast)
        a = vec_d[:]
        return bass.AP(tensor=a.tensor, offset=a.offset, ap=[[0, P], *a.ap])

    with tile.TileContext(nc) as tc:
        from contextlib import ExitStack, nullcontext

        # On-device benchmarking loop: rounds > 1 repeats the FULL kernel
        # (including every input DMA) back to back so per-round time can be
        # measured without the multi-ms per-dispatch tunnel overhead.
        loop_ctx = tc.For_i(0, rounds, 1) if rounds > 1 else nullcontext()
        with loop_ctx, ExitStack() as ctx:
            consts = ctx.enter_context(tc.tile_pool(name="consts", bufs=1))
            persist = ctx.enter_context(tc.tile_pool(name="persist", bufs=1))
            work = ctx.enter_context(tc.tile_pool(name="work", bufs=2))

            # ---- constants ----
            masksT_sb = []
            for qb in range(QB):
                m = consts.tile([P, NKT * P], BF16, tag=f"maskT{qb}",
                                name=f"maskT{qb}")
                nc.sync.dma_start(out=m, in_=masksT_d[qb])
                masksT_sb.append(m)
            ident = consts.tile([P, P], BF16, tag="ident", name="ident")
            nc.sync.dma_start(out=ident, in_=ident_d[:])
            bqs = consts.tile([P, ND], F32, tag="bqs", name="bqs")
            nc.sync.dma_start(out=bqs, in_=bqs_d[:])
            bks = consts.tile([P, ND], F32, tag="bks", name="bks")
            nc.sync.dma_start(out=bks, in_=bks_d[:])
            bv_bc = consts.tile([P, H * VP], BF16, tag="bv_bc", name="bv_bc")
            nc.sync.dma_start(out=bv_bc, in_=bcast(bv_d))
            if not ln_trivial:
                # these broadcast constants (512 KB DMA each per round) are
                # only read on the general-affine path
                bo_bc = consts.tile([P, D], F32, tag="bo_bc", name="bo_bc")
                nc.sync.dma_start(out=bo_bc, in_=bcast(bo_d))
                gamma_bc = consts.tile([P, D], F32, tag="gamma_bc",
                                       name="gamma_bc")
                nc.sync.dma_start(out=gamma_bc, in_=bcast(gamma_d))
                beta_bc = consts.tile([P, D], F32, tag="beta_bc",
                                      name="beta_bc")
                nc.sync.dma_start(out=beta_bc, in_=bcast(beta_d))
            ones_col = consts.tile([1, P], BF16, tag="ones_col",
                                   name="ones_col")
            nc.vector.memset(ones_col, 1.0)
            bgw_sb = consts.tile([1, D], BF16, tag="bgw", name="bgw")
            nc.sync.dma_start(out=bgw_sb, in_=bgw_d[:])
            magic_u = consts.tile([P, 1], U32, tag="magic", name="magic")
            if USE_NEWTON_RSQRT:
                nc.vector.memset(magic_u, RSQRT_MAGIC)
            eps_sb = consts.tile([P, 1], F32, tag="eps", name="eps")
            nc.vector.memset(eps_sb, LN_EPS)

            # ---- persistent activation tensors ----
            kT = [persist.tile([P, T], BF16, tag=f"kT{i}", name=f"kT{i}")
                  for i in range(ND)]
            # inner dim padded to VP=72 so each head slice starts 16B-aligned
            v_aug = [persist.tile([P, H, VP], BF16, tag=f"v{i}",
                                  name=f"v{i}") for i in range(NT)]
            qT = [persist.tile([P, T], BF16, tag=f"qT{i}", name=f"qT{i}")
                  for i in range(ND)]
            gate = [persist.tile([P, D], BF16, tag=f"g{i}", name=f"g{i}")
                    for i in range(NT)]

            # ---- input streaming: ALL major input DMAs issued up front so
            # PE never stalls on a pool-close barrier between phases.
            # pinA (cT, wkT, wvT) closes after phase 1 so woT can stream into
            # its space during Q-proj; pinB (hT, wqT) and pg (wgT) close
            # after the gate.
            # pool-stack (LIFO) order: pinA opened last of the input pools so
            # it can close right after phase 1; pinB/pg stay open to the end.
            pinB = ctx.enter_context(tc.tile_pool(name="pinB", bufs=1))
            pg = ctx.enter_context(tc.tile_pool(name="pg", bufs=1))
            pinA_ctx = tc.tile_pool(name="pinA", bufs=1)
            pinA = pinA_ctx.__enter__()

            c8_sb, wk8_sb, wv8_sb = [], [], []
            h8_sb, wq8_sb, wg8_sb = [], [], []
            # cT + wkT first: the kT matmuls need only these. Issued from the
            # (otherwise idle) Pool sequencer — SP's per-DMA config time
            # (~565 ns) serializes the stream badly when it issues all of
            # them itself.
            for p2 in range(ND2):
                t_ = pinA.tile([P, 2, T], F8, tag=f"c8_{p2}", name=f"c8_{p2}")
                nc.gpsimd.dma_start(out=t_, in_=c8_d[p2])
                c8_sb.append(t_)
                t_ = pinA.tile([P, 2, D], F8, tag=f"wk8_{p2}", name=f"wk8_{p2}")
                nc.gpsimd.dma_start(out=t_, in_=wk8_d[p2])
                wk8_sb.append(t_)
            for p2 in range(ND2):
                t_ = pinA.tile([P, 2, H * VP], F8, tag=f"wv8_{p2}",
                               name=f"wv8_{p2}")
                nc.sync.dma_start(out=t_, in_=wv8_d[p2])
                wv8_sb.append(t_)
                t_ = pinB.tile([P, 2, T], F8, tag=f"h8_{p2}", name=f"h8_{p2}")
                nc.sync.dma_start(out=t_, in_=h8_d[p2])
                h8_sb.append(t_)
            for p2 in range(ND2):
                t_ = pinB.tile([P, 2, D], F8, tag=f"wq8_{p2}", name=f"wq8_{p2}")
                nc.scalar.dma_start(out=t_, in_=wq8_d[p2])
                wq8_sb.append(t_)
                t_ = pg.tile([P, 2, D], F8, tag=f"wg8_{p2}", name=f"wg8_{p2}")
                nc.scalar.dma_start(out=t_, in_=wg8_d[p2])
                wg8_sb.append(t_)

            # ================= phase 1: K, V from cross =================
            ps12_ctx = tc.tile_pool(name="ps12", bufs=1, space="PSUM")
            ps12 = ps12_ctx.__enter__()

            for oc in range(ND):
                ps = ps12.tile([P, 2, 512], F32, tag="proj2", bufs=2,
                               name="ps_k")
                for th in range(2):
                    for p2 in range(ND2):
                        nc.tensor.matmul(
                            ps[:, th, :],
                            lhsT=wk8_sb[p2][:, :, oc * P:(oc + 1) * P],
                            rhs=c8_sb[p2][:, :, th * 512:(th + 1) * 512],
                            start=(p2 == 0), stop=(p2 == ND2 - 1),
                            perf_mode=PM_DR,
                        )
                nc.vector.tensor_scalar(
                    out=kT[oc], in0=ps.rearrange("p a b -> p (a b)"),
                    scalar1=1.0 / WS, scalar2=bks[:, oc:oc + 1],
                    op0=ALU.mult, op1=ALU.add,
                )

            # v_aug projection: 4 heads per matmul group (N = 4*VP = 288),
            # every elementwise op contiguous
            NVG = 4 * VP  # 288
            for tt in range(NT):
                for qg in range(4):
                    ps = ps12.tile([P, 512], F32, tag="proj", bufs=2,
                                   name="ps_v")
                    for p2 in range(ND2):
                        nc.tensor.matmul(
                            ps[:, 0:NVG],
                            lhsT=c8_sb[p2][:, :, tt * P:(tt + 1) * P],
                            rhs=wv8_sb[p2][:, :, qg * NVG:(qg + 1) * NVG],
                            start=(p2 == 0), stop=(p2 == ND2 - 1),
                            perf_mode=PM_DR,
                        )
                    nc.vector.scalar_tensor_tensor(
                        out=v_aug[tt][:, qg * 4:(qg + 1) * 4, :].rearrange(
                            "p a b -> p (a b)"),
                        in0=ps[:, 0:NVG], scalar=1.0 / WS,
                        in1=bv_bc[:, qg * NVG:(qg + 1) * NVG],
                        op0=ALU.mult, op1=ALU.add,
                    )

            # phase 1 inputs dead -> free their space; woT streams into it
            # while Q-proj runs
            pinA_ctx.__exit__(None, None, None)
            po = ctx.enter_context(tc.tile_pool(name="po", bufs=1))
            wo8_sb = []
            for p2 in range(ND2):
                t_ = po.tile([P, 2, D], F8, tag=f"wo8_{p2}", name=f"wo8_{p2}")
                nc.gpsimd.dma_start(out=t_, in_=wo8_d[p2])
                wo8_sb.append(t_)

            # ============ phase 2: Q, gate from hidden ============
            for oc in range(ND):
                ps = ps12.tile([P, 2, 512], F32, tag="proj2", bufs=2,
                               name="ps_q")
                for th in range(2):
                    for p2 in range(ND2):
                        nc.tensor.matmul(
                            ps[:, th, :],
                            lhsT=wq8_sb[p2][:, :, oc * P:(oc + 1) * P],
                            rhs=h8_sb[p2][:, :, th * 512:(th + 1) * 512],
                            start=(p2 == 0), stop=(p2 == ND2 - 1),
                            perf_mode=PM_DR,
                        )
                nc.vector.tensor_scalar(
                    out=qT[oc], in0=ps.rearrange("p a b -> p (a b)"),
                    scalar1=1.0 / WS, scalar2=bqs[:, oc:oc + 1],
                    op0=ALU.mult, op1=ALU.add,
                )

            for tt in range(NT):
                ps = ps12.tile([P, 2, 512], F32, tag="proj2", bufs=2,
                               name="ps_g")
                for oh in range(2):
                    for p2 in range(ND2):
                        nc.tensor.matmul(
                            ps[:, oh, :],
                            lhsT=h8_sb[p2][:, :, tt * P:(tt + 1) * P],
                            rhs=wg8_sb[p2][:, :, oh * 512:(oh + 1) * 512],
                            start=(p2 == 0), stop=False,
                            perf_mode=PM_DR,
                        )
                    # bias via rank-1 matmul (ones ⊗ WS*bg) so sigmoid can
                    # read the psum directly — saves a DVE pass
                    nc.tensor.matmul(
                        ps[:, oh, :],
                        lhsT=ones_col,
                        rhs=bgw_sb[:, oh * 512:(oh + 1) * 512],
                        start=False, stop=True, skip_group_check=True,
                    )
                nc.scalar.activation(
                    out=gate[tt], in_=ps.rearrange("p a b -> p (a b)"),
                    func=AF.Sigmoid, scale=1.0 / WS,
                )

            ps12_ctx.__exit__(None, None, None)

            # big phase-3 work tiles (fresh space; pinA's is used by po/w3)
            w3 = ctx.enter_context(tc.tile_pool(name="w3", bufs=3))

            # ===== phase 3: attention + out proj + epilogue =====
            with tc.tile_pool(name="ps3", bufs=1, space="PSUM") as ps3:
                if not BUILD_PHASE3:
                    for tt in range(NT):
                        h32t = w3.tile([P, D], F32, tag="h32t", name="h32t")
                        nc.sync.dma_start(out=h32t,
                                          in_=h32_d[tt * P:(tt + 1) * P, :])
                        ob = w3.tile([P, D], F32, tag="ob", name="ob")
                        nc.vector.tensor_add(out=ob, in0=h32t, in1=gate[tt])
                        nc.sync.dma_start(out=out_d[tt * P:(tt + 1) * P, :],
                                          in_=ob)
                    pass
                else:
                    for tt in range(NT):
                        s = tt // QB
                        qb = tt % QB
                        base_kt = (0 if qb < 2 else 1) + s * QB  # global key tile
                        # active window chunks: qb0's chunk 2 and qb3's chunk 0
                        # are entirely outside the band -> skip them everywhere
                        j0, j1 = ACTIVE_J[qb]
                        nj = j1 - j0

                        attn_sb = w3.tile([P, H, HD], BF16, tag="attn_sb",
                                            name=f"attn_sb{tt}")
                        for c in range(ND):
                            # scoresT for both heads of the pair: [k, hh, j, q]
                            # (inner dim 512 so each head slice is bank-aligned
                            # and the exp read stays within a single bank)
                            ps_sc = ps3.tile([P, 2, 512], F32, tag="sc", bufs=2,
                                             name="ps_sc")
                            for u in range(2):
                                h = 2 * c + u
                                row0 = (h % 2) * HD
                                for j in range(j0, j1):
                                    nc.tensor.matmul(
                                        ps_sc[:, u, j * P:(j + 1) * P],
                                        lhsT=kT[c][row0:row0 + HD,
                                                   (base_kt + j) * P:
                                                   (base_kt + j + 1) * P],
                                        rhs=qT[c][row0:row0 + HD,
                                                  tt * P:(tt + 1) * P],
                                        start=(j == j0), stop=False,
                                    )
                                # accumulate the additive band mask on PE:
                                # ident.T @ maskAddT == maskAddT
                                nc.tensor.matmul(
                                    ps_sc[:, u, j0 * P:j1 * P],
                                    lhsT=ident,
                                    rhs=masksT_sb[qb][:, j0 * P:j1 * P],
                                    start=False, stop=True,
                                    skip_group_check=True,
                                )
                            probsT = w3.tile([P, 2, NKT, P], BF16, tag="probsT",
                                               name="probsT", bufs=3)
                            # one exp per chunk over both heads: strided psum
                            # read crossing the bank boundary (ScE psum reads
                            # allow up to 4K free)
                            nc.scalar.activation(
                                out=probsT[:, :, j0:j1, :].rearrange(
                                    "p a b c -> p a (b c)"),
                                in_=ps_sc[:, :, j0 * P:j1 * P],
                                func=AF.Exp, scale=SCALE)
                            # attn + denominator via v_aug ones column.
                            # one PSUM tile per head: PE-write of head u=1 must
                            # not share a bank with DVE reads of head u=0
                            # (PSUM bank collisions are a hardware abort)
                            # both heads' PV into ONE psum bank ([P,2,VP]
                            # fits 576B < 2KB) so a single strided reciprocal
                            # covers both denominators. DVE reads wait for
                            # both PV groups (same bank as PE writes).
                            rden = work.tile([P, 2], F32, tag="rden", name="rden")
                            ps_aT = ps3.tile([P, 2, VP], F32, tag="aT",
                                             bufs=2, name="ps_aT")
                            for u in range(2):
                                h = 2 * c + u
                                for j in range(j0, j1):
                                    nc.tensor.matmul(
                                        ps_aT[:, u, 0:HD + 1],
                                        lhsT=probsT[:, u, j, :],
                                        rhs=v_aug[base_kt + j][:, h, 0:HD + 1],
                                        start=(j == j0), stop=(j == j1 - 1),
                                    )
                            nc.vector.reciprocal(out=rden,
                                                 in_=ps_aT[:, :, HD])
                            for u in range(2):
                                nc.vector.tensor_scalar_mul(
                                    out=attn_sb[:, 2 * c + u, :],
                                    in0=ps_aT[:, u, 0:HD],
                                    scalar1=rden[:, u:u + 1],
                                )

                        # transpose attn to feature-major (fp8, scaled by AS
                        # into fp8-normal range) for the Wo projection; the
                        # psum->sbuf copies ride the ACT engine, DVE is the
                        # attention-phase bottleneck
                        attnT = w3.tile([P, ND, P], F8, tag="attnT",
                                        name=f"attnT{tt}")
                        for c2 in range(ND // 2):
                            # two 128x128 transposes into one psum bank, one
                            # ACT copy for both
                            ps_tp = ps3.tile([P, 2, P], F32, tag="tp", bufs=2,
                                             name="ps_tp")
                            for w in range(2):
                                c = 2 * c2 + w
                                nc.tensor.matmul(
                                    ps_tp[:, w, :],
                                    lhsT=attn_sb[:, 2 * c:2 * c + 2, :],
                                    rhs=ident, start=True, stop=True,
                                )
                            nc.scalar.activation(
                                out=attnT[:, 2 * c2:2 * c2 + 2, :].rearrange(
                                    "p a b -> p (a b)"),
                                in_=ps_tp.rearrange("p a b -> p (a b)"),
                                func=AF.Identity, scale=AS)

                        # out projection + epilogue for this token tile
                        h32t = w3.tile([P, D], BF16, tag="h32t", name="h32t")
                        nc.sync.dma_start(out=h32t, in_=h32_d[tt * P:(tt + 1) * P, :])
                        ta = w3.tile([P, D], F32, tag="ta", name="ta")
                        for oh in range(2):
                            ps_o = ps3.tile([P, 512], F32, tag="sc", bufs=2,
                                            name="ps_o")
                            for p2 in range(ND2):
                                nc.tensor.matmul(
                                    ps_o,
                                    lhsT=attnT[:, 2 * p2:2 * p2 + 2, :],
                                    rhs=wo8_sb[p2][:, :, oh * 512:(oh + 1) * 512],
                                    start=(p2 == 0), stop=(p2 == ND2 - 1),
                                    perf_mode=PM_DR,
                                )
                            if ln_trivial:
                                # bo == 0 under the same trivial-affine init
                                nc.vector.tensor_scalar_mul(
                                    out=ta[:, oh * 512:(oh + 1) * 512],
                                    in0=ps_o, scalar1=1.0 / (WS * AS),
                                )
                            else:
                                nc.vector.scalar_tensor_tensor(
                                    out=ta[:, oh * 512:(oh + 1) * 512],
                                    in0=ps_o, scalar=1.0 / (WS * AS),
                                    in1=bo_bc[:, oh * 512:(oh + 1) * 512],
                                    op0=ALU.mult, op1=ALU.add,
                                )
                        # gated residual: pre = hidden + gate*attn (LN scale-inv)
                        tb = w3.tile([P, D], F32, tag="tb", name="tb")
                        nc.gpsimd.tensor_mul(out=ta, in0=ta, in1=gate[tt])
                        nc.gpsimd.tensor_add(out=tb, in0=ta, in1=h32t)
                        # LayerNorm stats
                        stats = work.tile([P, 2, 6], F32, tag="stats", name="stats")
                        for half in range(2):
                            nc.vector.bn_stats(out=stats[:, half, :],
                                               in_=tb[:, half * 512:(half + 1) * 512])
                        mv = work.tile([P, 2], F32, tag="mv", name="mv")
                        nc.vector.bn_aggr(out=mv, in_=stats)
                        # rstd = rsqrt(var + eps) via Newton on GpSimd (keeps the
                        # ACT engine's Exp table resident)
                        if USE_NEWTON_RSQRT:
                            xe = work.tile([P, 1], F32, tag="xe", name="xe")
                            nc.vector.tensor_scalar_add(out=xe, in0=mv[:, 1:2],
                                                        scalar1=LN_EPS)
                            yy = work.tile([P, 1], F32, tag="yy", name="yy")
                            tmp_u = work.tile([P, 1], U32, tag="tmp_u",
                                              name="tmp_u")
                            nc.vector.tensor_scalar(
                                out=tmp_u, in0=xe.bitcast(U32), scalar1=1,
                                scalar2=None, op0=ALU.logical_shift_right,
                            )
                            nc.vector.tensor_sub(out=yy.bitcast(U32),
                                                 in0=magic_u, in1=tmp_u)
                            t1 = work.tile([P, 1], F32, tag="nt1", name="nt1")
                            for _ in range(NEWTON_ITERS):
                                nc.vector.tensor_mul(out=t1, in0=yy, in1=yy)
                                nc.vector.tensor_mul(out=t1, in0=t1, in1=xe)
                                nc.vector.tensor_scalar(
                                    out=t1, in0=t1, scalar1=-0.5, scalar2=1.5,
                                    op0=ALU.mult, op1=ALU.add,
                                )
                                nc.vector.tensor_mul(out=yy, in0=yy, in1=t1)
                        else:
                            yy = work.tile([P, 1], F32, tag="yy", name="yy")
                            nc.scalar.activation(out=yy, in_=mv[:, 1:2],
                                                 func=AF.Sqrt, bias=eps_sb,
                                                 scale=1.0)
                            nc.vector.reciprocal(out=yy, in_=yy)
                        if ln_trivial:
                            # gamma==1, beta==0 (detected host-side):
                            # out = (tb - mean) * rstd in ONE fused DVE op
                            nc.vector.tensor_scalar(
                                out=ta, in0=tb, scalar1=mv[:, 0:1],
                                scalar2=yy, op0=ALU.subtract, op1=ALU.mult,
                            )
                            nc.sync.dma_start(
                                out=out_d[tt * P:(tt + 1) * P, :], in_=ta)
                        else:
                            # (tb-mean)*gamma -> ta; *rstd; + beta -> tb
                            nc.vector.scalar_tensor_tensor(
                                out=ta, in0=tb, scalar=mv[:, 0:1], in1=gamma_bc,
                                op0=ALU.subtract, op1=ALU.mult,
                            )
                            nc.vector.tensor_scalar_mul(out=ta, in0=ta,
                                                        scalar1=yy)
                            nc.gpsimd.tensor_add(out=tb, in0=ta, in1=beta_bc)
                            nc.sync.dma_start(
                                out=out_d[tt * P:(tt + 1) * P, :], in_=tb)

    nc.compile()
    return nc


def _pair8(M, scale=1.0):
    """[D, X] f32 -> fp8 DoubleRow layout [ND2, P, 2, X]."""
    X = M.shape[1]
    return np.ascontiguousarray(
        (np.asarray(M, np.float32) * scale).reshape(ND2, 2, P, X)
        .transpose(0, 2, 1, 3)).astype(ml_dtypes.float8_e4m3)


def _prep_host(inputs):
    bf = ml_dtypes.bfloat16
    hidden = np.ascontiguousarray(inputs["hidden_states"], dtype=np.float32)
    cross = np.ascontiguousarray(inputs["cross_states"], dtype=np.float32)
    wvT_aug = np.asarray(_augment_wv(inputs["Wv"]), dtype=np.float32)
    shared = {
        "wq8": _pair8(inputs["Wq"].T, WS),
        "wk8": _pair8(inputs["Wk"].T, WS),
        "wv8": _pair8(wvT_aug, WS),
        "wg8": _pair8(inputs["Wg"].T, WS),
        "wo8": _pair8(inputs["Wo"].T, WS),
        "bqs": np.ascontiguousarray(
            inputs["bq"].astype(np.float32).reshape(ND, P).T),
        "bks": np.ascontiguousarray(
            inputs["bk"].astype(np.float32).reshape(ND, P).T),
        "bv": _augment_bv(inputs["bv"]).astype(bf),
        "bgw": (inputs["bg"].astype(np.float32) * WS).reshape(1, D).astype(bf),
        "bo": inputs["bo"].astype(np.float32),
        "gamma": inputs["gamma"].astype(np.float32),
        "beta": inputs["beta"].astype(np.float32),
        "masksT": _build_masksT(),
        "ident": np.eye(P, dtype=bf),
    }
    in_maps = []
    for core in range(NCORES):
        hs = hidden[core * SEQ_PER_CORE:(core + 1) * SEQ_PER_CORE].reshape(T, D)
        cs = cross[core * SEQ_PER_CORE:(core + 1) * SEQ_PER_CORE].reshape(T, D)
        m = dict(shared)
        m["h32"] = np.ascontiguousarray(hs).astype(bf)
        m["h8"] = _pair8(hs.T)
        m["c8"] = _pair8(cs.T)
        in_maps.append(m)
    return in_maps


def _ln_trivial(inputs):
    return (np.allclose(np.asarray(inputs["gamma"], np.float32), 1.0) and
            np.allclose(np.asarray(inputs["beta"], np.float32), 0.0) and
            np.allclose(np.asarray(inputs["bo"], np.float32), 0.0))


def _run(inputs, trace=False):
    lt = _ln_trivial(inputs)
    key = f"nc1_{lt}"
    if key not in _CACHE:
        _CACHE[key] = _build_program(rounds=1, ln_trivial=lt)
    nc = _CACHE[key]
    in_maps = _prep_host(inputs)
    res = run_bass_kernel_spmd(nc, in_maps, list(range(NCORES)), trace=trace)
    out = np.empty((B, S, D), dtype=np.float32)
    for core in range(NCORES):
        out[core * SEQ_PER_CORE:(core + 1) * SEQ_PER_CORE] = (
            np.asarray(res.results[core]["out"], dtype=np.float32).reshape(
                SEQ_PER_CORE, S, D))
    return out, res


def kernel(**inputs):
    out, _ = _run(inputs, trace=False)
    return out


def bench(inputs, iters=20, rounds=1):
    """Amortized device-time benchmark: device-resident inputs, N back-to-back
    dispatches of a program that runs the full kernel `rounds` times
    on-device, report per-round wall time."""
    import time

    import jax
    from jax.sharding import Mesh, NamedSharding, PartitionSpec
    from jax.experimental.shard_map import shard_map
    from concourse import bass2jax, mybir as _mybir

    lt = _ln_trivial(inputs)
    key = f"nc{rounds}_{lt}"
    if key not in _CACHE:
        _CACHE[key] = _build_program(rounds=rounds, ln_trivial=lt)
    nc = _CACHE[key]
    in_maps = _prep_host(inputs)
    bass2jax.install_neuronx_cc_hook()

    partition_name = (nc.partition_id_tensor.name if nc.partition_id_tensor
                      else None)
    in_names, out_names, out_avals, zero_outs = [], [], [], []
    for alloc in nc.m.functions[0].allocations:
        if not isinstance(alloc, _mybir.MemoryLocationSet):
            continue
        name = alloc.memorylocations[0].name
        if alloc.kind == "ExternalInput":
            if name != partition_name:
                in_names.append(name)
        elif alloc.kind == "ExternalOutput":
            out_names.append(name)
            shape = tuple(alloc.tensor_shape)
            dtype = _mybir.dt.np(alloc.dtype)
            out_avals.append(jax.core.ShapedArray(shape, dtype))
            zero_outs.append(np.zeros(shape, dtype))
    n_params = len(in_names)
    all_in_names = in_names + out_names
    if partition_name is not None:
        all_in_names.append(partition_name)

    def _body(*args):
        operands = list(args)
        if partition_name is not None:
            operands.append(bass2jax.partition_id_tensor())
        outs = bass2jax._bass_exec_p.bind(
            *operands,
            out_avals=tuple(out_avals),
            in_names=tuple(all_in_names),
            out_names=tuple(out_names),
            lowering_input_output_aliases=(),
            sim_require_finite=True,
            sim_require_nnan=True,
            nc=nc,
        )
        return tuple(outs)

    devices = jax.devices()[:NCORES]
    mesh = Mesh(np.asarray(devices), ("core",))
    spec = PartitionSpec("core")
    n_outs = len(out_names)
    sharded = jax.jit(
        shard_map(_body, mesh=mesh, in_specs=(spec,) * (n_params + n_outs),
                  out_specs=(spec,) * n_outs, check_rep=False),
        keep_unused=True,
    )
    concat_in = [
        np.concatenate([np.asarray(in_maps[c][name]) for c in range(NCORES)],
                       axis=0)
        for name in in_names
    ]
    concat_zero = [np.zeros((NCORES * z.shape[0], *z.shape[1:]), z.dtype)
                   for z in zero_outs]
    sh = NamedSharding(mesh, spec)
    dev_in = [jax.device_put(a, sh) for a in concat_in]
    dev_zero = [jax.device_put(a, sh) for a in concat_zero]

    # warmup (compile)
    out = sharded(*dev_in, *dev_zero)
    jax.block_until_ready(out)
    t0 = time.perf_counter()
    for _ in range(iters):
        out = sharded(*dev_in, *dev_zero)
    jax.block_until_ready(out)
    t1 = time.perf_counter()
    per_round_ns = (t1 - t0) / (iters * rounds) * 1e9
    return per_round_ns, out

